# revision 3
# baseline (speedup 1.0000x reference)
"""GCN VGAE encoder (3x GCNConv) on 8 Trainium2 NeuronCores.

Strategy (per spec sharding hint): shard nodes across 8 cores, partition
edges by destination node (host-side, a byproduct of the 1D graph
partitioning), replicate weights, AllGather the projected+scaled node
feature table between layers, gather halo features with dma_gather.

Math: A_hat = D^-1/2 (A+I) D^-1/2 factorizes, so
    gcn(x, W) = dinv * [ (x@W)*dinv + A @ ((x@W)*dinv) ] + b
with dinv = 1/sqrt(deg+1) per node. Per-edge work is pure gather +
scatter-add of 64-float rows; scatter-add is a one-hot matmul on PE
(edges sorted by destination tile, PSUM accumulation per tile).
The two output convs share the adjacency, so W_mu|W_log are fused into
one 64-wide layer-2 table pass.
"""

import numpy as np

P = 128


def _ceil_div(a, b):
    return -(-a // b)


class _Plan:
    """Host-side edge partitioning shared by all cores (SPMD => one
    common chunk structure = max over cores, padded)."""

    def __init__(self, n, n_cores, cpc, src, dst, max_slice_rows=32000):
        assert n % n_cores == 0
        self.n = n
        self.n_cores = n_cores
        self.cpc = cpc                     # chunks per dma_gather call
        self.S = n // n_cores              # nodes per core
        self.T = _ceil_div(self.S, P)      # dst tiles per core
        self.SPAD = self.T * P
        # quarters: gather index is int16 -> table slice rows <= 32000
        nq = 1
        while _ceil_div(n, nq) > max_slice_rows:
            nq *= 2
        self.NQ = nq
        self.QR = _ceil_div(n, nq)         # rows per table slice

        core = dst // self.S
        drel = dst - core * self.S
        tt = drel // P
        loc = (drel % P).astype(np.float32)
        q = src // self.QR
        qsrc = (src - q * self.QR).astype(np.int16)

        T, NQ = self.T, self.NQ
        key = (core * NQ + q) * T + tt
        counts = np.bincount(key, minlength=n_cores * NQ * T).reshape(
            n_cores, NQ, T
        )
        # common run length per (quarter, tile): max over cores, runs are
        # packed back-to-back in the quarter stream (no 128-padding per
        # run; chunks may span adjacent tiles).
        self.rl = counts.max(axis=0)                         # [NQ, T]
        self.run_start = np.zeros((NQ, T), np.int64)
        self.run_start[:, 1:] = np.cumsum(self.rl, axis=1)[:, :-1]
        self.NQE = self.rl.sum(axis=1)                       # edges/quarter
        self.NQC = _ceil_div(self.NQE, P)                    # chunks/quarter
        self.NCH = int(self.NQC.sum())

        # order edges by (core, quarter, tile); rank within group
        sidx = np.lexsort((tt, q, core))
        self.sc = core[sidx]
        self.sq = q[sidx]
        self.st = tt[sidx]
        self.sqsrc = qsrc[sidx]
        self.sloc = loc[sidx]
        gkey = (self.sc * NQ + self.sq) * T + self.st
        first = np.r_[True, gkey[1:] != gkey[:-1]]
        gstart = np.flatnonzero(first)
        glen = np.diff(np.r_[gstart, len(gkey)])
        self.rank = np.arange(len(gkey)) - np.repeat(gstart, glen)

        # chunk-part (cp) map: device consumes tiles in order; for tile t
        # and quarter q, the run covers chunks j0..j1 of quarter q's
        # stream; each (t, q, j) overlap gets its own dstloc column.
        self.tile_ops = []        # [T] -> list of (q, j, cp_col)
        self.cp_of = {}           # (q, j, t) -> cp column
        ncp = 0
        for t in range(T):
            ops = []
            for qq in range(NQ):
                r0 = int(self.run_start[qq, t])
                r1 = r0 + int(self.rl[qq, t])
                if r1 == r0:
                    continue
                for j in range(r0 // P, (r1 - 1) // P + 1):
                    ops.append((qq, j, ncp))
                    self.cp_of[(qq, j, t)] = ncp
                    ncp += 1
            self.tile_ops.append(ops)
        self.NCP = ncp
        # vectorized cp lookup: cp = cp_base[t] + ops_before[q,t] + (j - j0)
        self.cp_base = np.zeros(T, np.int64)
        run2 = 0
        self.ops_before = np.zeros((NQ, T), np.int64)
        self.j0 = self.run_start // P
        for t in range(T):
            self.cp_base[t] = run2
            acc = 0
            for qq in range(NQ):
                self.ops_before[qq, t] = acc
                if self.rl[qq, t] > 0:
                    r0 = int(self.run_start[qq, t])
                    r1 = r0 + int(self.rl[qq, t])
                    acc += (r1 - 1) // P - r0 // P + 1
            run2 += acc
        assert run2 == ncp

        # gather calls per quarter
        self.ncalls = [_ceil_div(int(c), cpc) for c in self.NQC]
        # idx tensor column offset of each (quarter, call)
        self.call_col0 = {}
        col = 0
        for qq in range(NQ):
            for k in range(self.ncalls[qq]):
                L = min(cpc, int(self.NQC[qq]) - k * cpc)
                self.call_col0[(qq, k)] = (col, L)
                col += L * 8
        self.IDXCOLS = col

    def core_arrays(self, c, dst, n):
        """Per-core upload tensors: gather idx [128, IDXCOLS] i16,
        dstloc [128, NCP] f32, deg cols [128, T] f32."""
        NQ, T, cpc = self.NQ, self.T, self.cpc
        m_core = self.sc == c
        idx_out = np.zeros((P, self.IDXCOLS), np.int16)
        dl = np.full((self.NCP, P), 255.0, np.float32)
        mloc = self.sloc[m_core]
        mq = self.sq[m_core]
        mt = self.st[m_core]
        mrank = self.rank[m_core]
        msrc = self.sqsrc[m_core]
        # stream position of each edge within its quarter
        pos = self.run_start[mq, mt] + mrank
        cpcol = (self.cp_base[mt] + self.ops_before[mq, mt]
                 + pos // P - self.j0[mq, mt])
        dl[cpcol, pos % P] = mloc
        for qq in range(NQ):
            mm = mq == qq
            arr = np.zeros(int(self.NQC[qq]) * P, np.int16)
            arr[pos[mm]] = msrc[mm]
            for k in range(self.ncalls[qq]):
                c0, L = self.call_col0[(qq, k)]
                seg = arr[k * cpc * P:(k * cpc + L) * P]
                wrapped = seg.reshape(L * 8, 16).T       # [16, L*8]
                idx_out[:, c0:c0 + L * 8] = np.tile(wrapped, (8, 1))
        deg = np.bincount(dst, minlength=n)[c * self.S:(c + 1) * self.S]
        degp = np.zeros(self.SPAD, np.float32)
        degp[:self.S] = deg
        return idx_out, dl.T.copy(), degp.reshape(self.T, P).T.copy()


def _build(plan, d_in, d_h, d_o):
    """Build the SPMD Bass program (same for every core)."""
    import concourse.mybir as mybir
    import concourse.tile as tile
    from concourse import bacc
    from concourse.masks import make_identity

    F32 = mybir.dt.float32
    I16 = mybir.dt.int16
    n, T, NQ, SPAD, S, QR = plan.n, plan.T, plan.NQ, plan.SPAD, plan.S, plan.QR
    NCH, cpc = plan.NCH, plan.cpc
    n_cores = plan.n_cores

    nc = bacc.Bacc("TRN2", target_bir_lowering=False,
                   debug=False, num_devices=n_cores)

    x_d = nc.dram_tensor("x", [SPAD, d_in], F32, kind="ExternalInput")
    w1_d = nc.dram_tensor("w1", [d_in, d_h], F32, kind="ExternalInput")
    wc_d = nc.dram_tensor("wcat", [d_h, d_o], F32, kind="ExternalInput")
    b1_d = nc.dram_tensor("b1", [d_h], F32, kind="ExternalInput")
    bc_d = nc.dram_tensor("bcat", [d_o], F32, kind="ExternalInput")
    deg_d = nc.dram_tensor("deg", [P, T], F32, kind="ExternalInput")
    dl_d = nc.dram_tensor("dstloc", [P, plan.NCP], F32, kind="ExternalInput")
    idx_d = nc.dram_tensor("gidx", [P, plan.IDXCOLS], I16, kind="ExternalInput")
    out_d = nc.dram_tensor("out2", [SPAD, d_o], F32, kind="ExternalOutput")

    t1s_own = nc.dram_tensor("t1s_own", [S, d_h], F32, kind="Internal")
    t1s_full = nc.dram_tensor("t1s_full", [n, d_h], F32, kind="Internal",
                              addr_space="Shared")
    z2s_own = nc.dram_tensor("z2s_own", [S, d_o], F32, kind="Internal")
    z2s_full = nc.dram_tensor("z2s_full", [n, d_o], F32, kind="Internal",
                              addr_space="Shared")
    rg = [list(range(n_cores))]

    from contextlib import ExitStack

    with tile.TileContext(nc, num_cores=n_cores) as tc, ExitStack() as st:
        cp = st.enter_context(tc.tile_pool(name="consts", bufs=1))
        bigp = st.enter_context(tc.tile_pool(name="big", bufs=1))
        xp = st.enter_context(tc.tile_pool(name="x", bufs=3))
        xtp = st.enter_context(tc.tile_pool(name="xt", bufs=2))
        htp = st.enter_context(tc.tile_pool(name="ht", bufs=2))
        ohp = st.enter_context(tc.tile_pool(name="oh", bufs=8))
        gps = [st.enter_context(tc.tile_pool(name=f"g{q}", bufs=3))
               for q in range(NQ)]
        mmp = st.enter_context(tc.tile_pool(name="mm", bufs=4, space="PSUM"))
        aggp = st.enter_context(tc.tile_pool(name="agg", bufs=4,
                                             space="PSUM"))

        # ---- constants ----
        iota_i = cp.tile([P, P], mybir.dt.int32)
        nc.gpsimd.iota(iota_i[:], pattern=[[1, P]], base=0,
                       channel_multiplier=0)
        iota_f = cp.tile([P, P], F32)
        nc.vector.tensor_copy(iota_f[:], iota_i[:])
        ident = cp.tile([P, P], F32)
        make_identity(nc, ident[:])
        ones_row = cp.tile([1, P], F32)
        nc.gpsimd.memset(ones_row[:], 1.0)

        w1_sb = cp.tile([d_in, d_h], F32)
        nc.sync.dma_start(w1_sb[:], w1_d[:, :])
        wc_sb = cp.tile([d_h, d_o], F32)
        nc.sync.dma_start(wc_sb[:], wc_d[:, :])
        b1r = cp.tile([1, d_h], F32)
        nc.sync.dma_start(b1r[:], b1_d[None, :])
        bcr = cp.tile([1, d_o], F32)
        nc.sync.dma_start(bcr[:], bc_d[None, :])

        # bias rows broadcast to 128 partitions via ones-matmul
        b1bc = cp.tile([P, d_h], F32)
        ps = mmp.tile([P, P], F32, space="PSUM", tag="mm")
        nc.tensor.matmul(ps[:, :d_h], lhsT=ones_row[:], rhs=b1r[:],
                         start=True, stop=True)
        nc.vector.tensor_copy(b1bc[:], ps[:, :d_h])
        bcbc = cp.tile([P, d_o], F32)
        ps = mmp.tile([P, P], F32, space="PSUM", tag="mm")
        nc.tensor.matmul(ps[:, :d_o], lhsT=ones_row[:], rhs=bcr[:],
                         start=True, stop=True)
        nc.vector.tensor_copy(bcbc[:], ps[:, :d_o])

        deg_sb = cp.tile([P, T], F32)
        nc.sync.dma_start(deg_sb[:], deg_d[:, :])
        sq_sb = cp.tile([P, T], F32)
        # sqrt(deg + 1): +1 is the self-loop
        nc.scalar.activation(sq_sb[:], deg_sb[:],
                             mybir.ActivationFunctionType.Sqrt,
                             bias=1.0, scale=1.0)
        dinv = cp.tile([P, T], F32)
        nc.vector.reciprocal(dinv[:], sq_sb[:])

        dl_sb = cp.tile([P, plan.NCP], F32)
        nc.sync.dma_start(dl_sb[:], dl_d[:, :])
        idx_sb = cp.tile([P, plan.IDXCOLS], I16)
        nc.sync.dma_start(idx_sb[:], idx_d[:, :])

        t1s_sb = bigp.tile([P, T, d_h], F32)
        z2s_sb = bigp.tile([P, T, d_o], F32)

        # ---- layer-1 projection: t1s = (x @ W1) * dinv ----
        for t in range(T):
            xt = xp.tile([P, d_in], F32)
            nc.sync.dma_start(xt[:], x_d[t * P:(t + 1) * P, :])
            pst = mmp.tile([P, P], F32, space="PSUM", tag="mm")
            nc.tensor.transpose(pst[:d_in, :], xt[:], ident[:])
            xT = xtp.tile([d_in, P], F32)
            nc.vector.tensor_copy(xT[:], pst[:d_in, :])
            psm = mmp.tile([P, P], F32, space="PSUM", tag="mm")
            nc.tensor.matmul(psm[:, :d_h], lhsT=xT[:], rhs=w1_sb[:],
                             start=True, stop=True)
            nc.vector.tensor_scalar(t1s_sb[:, t, :], psm[:, :d_h],
                                    dinv[:, t:t + 1], None,
                                    mybir.AluOpType.mult)
            r0 = t * P
            r1 = min(S, r0 + P)
            if r1 > r0:
                nc.sync.dma_start(t1s_own[r0:r1, :], t1s_sb[:r1 - r0, t, :])

        import os as _os2
        _nocoll = bool(_os2.environ.get("GCN_NOCOLL"))
        if _nocoll:
            nc.sync.dma_start(t1s_full[0:S, :], t1s_own[:, :])
        else:
            nc.gpsimd.collective_compute(
                "AllGather", mybir.AluOpType.bypass, replica_groups=rg,
                ins=[t1s_own[:, :].opt()], outs=[t1s_full[:, :].opt()])

        def agg_pass(table, acc_sb, d_f):
            """acc_sb[:, t, :] += sum_e onehot(dst) * table[src]."""
            issued = {}

            def get_call(qq, k):
                if (qq, k) not in issued:
                    c0, L = plan.call_col0[(qq, k)]
                    g = gps[qq].tile([P, cpc, d_f], F32, tag=f"gt{qq}")
                    q0 = qq * QR
                    q1 = min(n, q0 + QR)
                    nc.gpsimd.dma_gather(
                        out_ap=g[:, :L, :],
                        in_ap=table[q0:q1, :],
                        idxs_ap=idx_sb[:, c0:c0 + L * 8],
                        num_idxs=L * P,
                        num_idxs_reg=L * P,
                        elem_size=d_f,
                        single_packet=False,
                    )
                    issued[(qq, k)] = g
                return issued[(qq, k)]

            for t in range(T):
                ops = plan.tile_ops[t]
                if not ops:
                    continue
                psa = aggp.tile([P, d_f], F32, space="PSUM", tag="agg")
                for i, (qq, j, cpcol) in enumerate(ops):
                    g = get_call(qq, j // cpc)
                    col = j % cpc
                    oh = ohp.tile([P, P], F32, tag="oh")
                    nc.vector.tensor_scalar(
                        oh[:], iota_f[:], dl_sb[:, cpcol:cpcol + 1], None,
                        mybir.AluOpType.is_equal)
                    nc.tensor.matmul(psa[:], lhsT=oh[:],
                                     rhs=g[:, col, :],
                                     start=(i == 0),
                                     stop=(i == len(ops) - 1))
                nc.vector.tensor_tensor(acc_sb[:, t, :], acc_sb[:, t, :],
                                        psa[:], mybir.AluOpType.add)

        import os as _os
        _stage = int(_os.environ.get("GCN_STAGE", "2"))
        if _stage in (1, 2):
            agg_pass(t1s_full, t1s_sb, d_h)

        # ---- h = relu(agg1 * dinv + b1); z2s = (h @ Wcat) * dinv ----
        for t in range(T):
            nc.vector.scalar_tensor_tensor(
                t1s_sb[:, t, :], t1s_sb[:, t, :], dinv[:, t:t + 1],
                b1bc[:], mybir.AluOpType.mult, mybir.AluOpType.add)
            nc.scalar.activation(t1s_sb[:, t, :], t1s_sb[:, t, :],
                                 mybir.ActivationFunctionType.Relu)
            pst = mmp.tile([P, P], F32, space="PSUM", tag="mm")
            nc.tensor.transpose(pst[:d_h, :], t1s_sb[:, t, :], ident[:])
            hT = htp.tile([d_h, P], F32)
            nc.vector.tensor_copy(hT[:], pst[:d_h, :])
            psm = mmp.tile([P, P], F32, space="PSUM", tag="mm")
            nc.tensor.matmul(psm[:, :d_o], lhsT=hT[:], rhs=wc_sb[:],
                             start=True, stop=True)
            nc.vector.tensor_scalar(z2s_sb[:, t, :], psm[:, :d_o],
                                    dinv[:, t:t + 1], None,
                                    mybir.AluOpType.mult)
            r0 = t * P
            r1 = min(S, r0 + P)
            if r1 > r0:
                nc.sync.dma_start(z2s_own[r0:r1, :], z2s_sb[:r1 - r0, t, :])

        if _nocoll:
            nc.sync.dma_start(z2s_full[0:S, :], z2s_own[:, :])
        else:
            nc.gpsimd.collective_compute(
                "AllGather", mybir.AluOpType.bypass, replica_groups=rg,
                ins=[z2s_own[:, :].opt()], outs=[z2s_full[:, :].opt()])

        if _stage in (2, 3):
            agg_pass(z2s_full, z2s_sb, d_o)

        # ---- out2 = agg2 * dinv + bcat ----
        for t in range(T):
            nc.vector.scalar_tensor_tensor(
                z2s_sb[:, t, :], z2s_sb[:, t, :], dinv[:, t:t + 1],
                bcbc[:], mybir.AluOpType.mult, mybir.AluOpType.add)
            nc.sync.dma_start(out_d[t * P:(t + 1) * P, :], z2s_sb[:, t, :])

    nc.compile()
    return nc


_CACHE = {}


def _get_program(n, e, d_in, d_h, d_o, n_cores, cpc, edge_key, src, dst,
                 max_slice_rows=32000):
    key = (n, e, d_in, d_h, d_o, n_cores, cpc, edge_key, max_slice_rows)
    if key not in _CACHE:
        plan = _Plan(n, n_cores, cpc, src, dst, max_slice_rows)
        nc = _build(plan, d_in, d_h, d_o)
        _CACHE[key] = (plan, nc)
    return _CACHE[key]


def kernel(x, edge_index, W1, b1, W_mu, b_mu, W_log, b_log,
           n_cores=8, cpc=16, max_slice_rows=32000, _run_kwargs=None):
    from concourse.bass_utils import run_bass_kernel_spmd

    x = np.asarray(x, np.float32)
    edge_index = np.asarray(edge_index)
    W1 = np.asarray(W1, np.float32)
    Wcat = np.concatenate([np.asarray(W_mu, np.float32),
                           np.asarray(W_log, np.float32)], axis=1)
    bcat = np.concatenate([np.asarray(b_mu, np.float32),
                           np.asarray(b_log, np.float32)])
    b1 = np.asarray(b1, np.float32)
    n, d_in = x.shape
    d_h = W1.shape[1]
    d_o = Wcat.shape[1]
    lat = np.asarray(W_mu, np.float32).shape[1]
    src = edge_index[0].astype(np.int64)
    dst = edge_index[1].astype(np.int64)

    edge_key = hash((src.tobytes(), dst.tobytes()))
    plan, nc = _get_program(n, len(src), d_in, d_h, d_o, n_cores, cpc,
                            edge_key, src, dst, max_slice_rows)

    in_maps = []
    for c in range(n_cores):
        idx_u, dl, deg = plan.core_arrays(c, dst, n)
        xs = np.zeros((plan.SPAD, d_in), np.float32)
        xs[:plan.S] = x[c * plan.S:(c + 1) * plan.S]
        in_maps.append({
            "x": xs, "w1": W1, "wcat": Wcat, "b1": b1, "bcat": bcat,
            "deg": deg, "dstloc": dl, "gidx": idx_u,
        })

    global _LAST_RESULT, _LAST_IN_MAPS
    _LAST_IN_MAPS = in_maps
    res = run_bass_kernel_spmd(nc, in_maps, core_ids=list(range(n_cores)),
                               **(_run_kwargs or {}))
    _LAST_RESULT = res
    out = np.concatenate(
        [res.results[c]["out2"][:plan.S] for c in range(n_cores)], axis=0)
    return (out[:, :lat].copy(), out[:, lat:].copy())


_LAST_RESULT = None
_LAST_IN_MAPS = None



# revision 8
# speedup vs baseline: 1.7836x; 1.7836x over previous
"""GCN VGAE encoder (3x GCNConv) on 8 Trainium2 NeuronCores.

Strategy: shard nodes across 8 cores, partition edges by destination
node (host-side 1D graph partitioning), replicate weights.

Math: A_hat = D^-1/2 (A+I) D^-1/2, dinv = 1/sqrt(deg+1). Aggregation
commutes with the linear projections, so layer 1 aggregates RAW x rows
(gathered bf16 from a replicated DRAM table -- no AllGather needed) and
projects afterwards. The per-edge norm w_e = dinv[src]*dinv[dst] is
folded into the one-hot scatter weights (host-computed), and the self
loop contributes dinv^2 * row analytically. Layers 2/3 share the
adjacency, so W_mu|W_log fuse into one 64-wide pass over the z2 table
(f32 DRAM table for the 256B gather constraint, cast to bf16 on-chip
for 4x PE throughput). One-hot matrices are built 16 chunks at a time
with stride-0 broadcast APs (2 DVE instructions per gather call).
"""

import numpy as np
import ml_dtypes

P = 128
BF = ml_dtypes.bfloat16


def _ceil_div(a, b):
    return -(-a // b)


class _Plan:
    """Host-side edge partitioning shared by all cores (SPMD => one
    common padded chunk structure = max over cores).

    Edges are grouped by (dst core, src quarter, dst tile); each (q,t)
    run is padded to a multiple of 128 so chunks never span tiles.
    Quarter q's edge stream is the concatenation of its runs over t.
    """

    def __init__(self, n, n_cores, cpc, src, dst, qr=25000):
        assert n % n_cores == 0
        self.n = n
        self.n_cores = n_cores
        self.cpc = cpc
        self.S = n // n_cores
        self.T = _ceil_div(self.S, P)
        self.SPAD = self.T * P
        self.NQ = _ceil_div(n, qr)
        self.QR = qr

        core = dst // self.S
        drel = dst - core * self.S
        tt = drel // P
        loc = (drel % P).astype(np.float32)
        q = src // qr
        qsrc = (src - q * qr).astype(np.int16)

        T, NQ = self.T, self.NQ
        counts = np.bincount((core * NQ + q) * T + tt,
                             minlength=n_cores * NQ * T).reshape(
            n_cores, NQ, T)
        rl = counts.max(axis=0)                       # [NQ, T]
        self.nch_qt = _ceil_div(rl, P)                # chunks per (q,t)
        PL = self.nch_qt * P                          # padded run length
        qt_start = np.zeros((NQ, T), np.int64)
        qt_start[:, 1:] = np.cumsum(PL, axis=1)[:, :-1]
        self.qt_start = qt_start
        self.NQC = PL.sum(axis=1) // P                # chunks per quarter
        self.NCH = int(self.NQC.sum())
        self.qcol0 = np.zeros(NQ + 1, np.int64)
        self.qcol0[1:] = np.cumsum(self.NQC)
        self.chunk0 = qt_start // P                   # [NQ, T]

        # order edges by (core, quarter, tile, src) -- src-sorted runs
        # give the gather DMA ascending addresses (DRAM locality).
        sidx = np.lexsort((src, tt, q, core))
        self.sc = core[sidx]
        self.sq = q[sidx]
        self.st = tt[sidx]
        self.sqsrc = qsrc[sidx]
        self.sloc = loc[sidx]
        self.ssrc = src[sidx]
        self.sdst = dst[sidx]
        gkey = (self.sc * NQ + self.sq) * T + self.st
        first = np.r_[True, gkey[1:] != gkey[:-1]]
        gstart = np.flatnonzero(first)
        glen = np.diff(np.r_[gstart, len(gkey)])
        self.rank = np.arange(len(gkey)) - np.repeat(gstart, glen)

        self.ncalls = [_ceil_div(int(c), cpc) for c in self.NQC]
        self.IDXCOLS = self.NCH * 8

    def core_arrays(self, c, dinv):
        """Per-core tensors: gather idx [P, IDXCOLS] i16, dl [P, NCH]
        bf16, w [P, NCH] bf16, dinv2 [P, T] f32."""
        NQ, cpc, NCH = self.NQ, self.cpc, self.NCH
        m = self.sc == c
        mq = self.sq[m]
        mt = self.st[m]
        mrank = self.rank[m]
        msrc = self.sqsrc[m]
        mloc = self.sloc[m]
        mw = (dinv[self.ssrc[m]] * dinv[self.sdst[m]]).astype(np.float32)

        pos = self.qt_start[mq, mt] + mrank           # slot in q stream
        ccol = self.qcol0[mq] + pos // P              # global chunk col
        dl = np.full((NCH, P), 255.0, np.float32)
        w = np.zeros((NCH, P), np.float32)
        dl[ccol, pos % P] = mloc
        w[ccol, pos % P] = mw

        idx_out = np.zeros((P, self.IDXCOLS), np.int16)
        for qq in range(NQ):
            nqc = int(self.NQC[qq])
            if nqc == 0:
                continue
            arr = np.zeros(nqc * P, np.int16)
            mm = mq == qq
            arr[pos[mm]] = msrc[mm]
            c0 = int(self.qcol0[qq]) * 8
            wrapped = arr.reshape(nqc * 8, 16).T      # [16, nqc*8]
            idx_out[:, c0:c0 + nqc * 8] = np.tile(wrapped, (8, 1))

        d2 = np.zeros(self.SPAD, np.float32)
        d2[:self.S] = dinv[c * self.S:(c + 1) * self.S] ** 2
        return (idx_out, dl.T.astype(BF).copy(), w.T.astype(BF).copy(),
                d2.reshape(self.T, P).T.copy())


def _build(plan, d_in, d_h, d_o):
    """Build the SPMD Bass program (same for every core)."""
    import concourse.mybir as mybir
    import concourse.tile as tile
    from concourse import bacc
    from concourse.masks import make_identity
    from contextlib import ExitStack
    import os

    F32 = mybir.dt.float32
    BF16 = mybir.dt.bfloat16
    I16 = mybir.dt.int16
    AF = mybir.ActivationFunctionType
    OP = mybir.AluOpType
    n, T, NQ, SPAD, S, QR = plan.n, plan.T, plan.NQ, plan.SPAD, plan.S, plan.QR
    cpc = plan.cpc
    n_cores = plan.n_cores
    _stage = int(os.environ.get("GCN_STAGE", "2"))
    _nocoll = bool(os.environ.get("GCN_NOCOLL"))

    nc = bacc.Bacc("TRN2", target_bir_lowering=False,
                   debug=False, num_devices=n_cores)

    xtab_d = nc.dram_tensor("xtab", [n, d_in], BF16, kind="ExternalInput")
    xown_d = nc.dram_tensor("xown", [SPAD, d_in], BF16, kind="ExternalInput")
    w1_d = nc.dram_tensor("w1", [d_in, d_h], BF16, kind="ExternalInput")
    wc_d = nc.dram_tensor("wcat", [d_h, d_o], BF16, kind="ExternalInput")
    b1_d = nc.dram_tensor("b1", [1, d_h], BF16, kind="ExternalInput")
    bc_d = nc.dram_tensor("bcat", [1, d_o], BF16, kind="ExternalInput")
    d2_d = nc.dram_tensor("dinv2", [P, T], F32, kind="ExternalInput")
    dl_d = nc.dram_tensor("dl", [P, plan.NCH], BF16, kind="ExternalInput")
    w_d = nc.dram_tensor("w", [P, plan.NCH], BF16, kind="ExternalInput")
    idx_d = nc.dram_tensor("gidx", [P, plan.IDXCOLS], I16,
                           kind="ExternalInput")
    out_d = nc.dram_tensor("out2", [SPAD, d_o], F32, kind="ExternalOutput")

    z2s_own = nc.dram_tensor("z2s_own", [S, d_o], F32, kind="Internal")
    z2s_full = nc.dram_tensor("z2s_full", [n, d_o], F32, kind="Internal",
                              addr_space="Shared")
    rg = [list(range(n_cores))]

    with tile.TileContext(nc, num_cores=n_cores) as tc, ExitStack() as st:
        cp = st.enter_context(tc.tile_pool(name="consts", bufs=1))
        bigp = st.enter_context(tc.tile_pool(name="big", bufs=1))
        gp = st.enter_context(tc.tile_pool(name="gath", bufs=3))
        gbp = st.enter_context(tc.tile_pool(name="gbf", bufs=3))
        ohp = st.enter_context(tc.tile_pool(name="oh", bufs=3))
        xop = st.enter_context(tc.tile_pool(name="xo", bufs=3))
        tp = st.enter_context(tc.tile_pool(name="stage", bufs=4))
        mmp = st.enter_context(tc.tile_pool(name="mm", bufs=4, space="PSUM"))
        aggp = st.enter_context(tc.tile_pool(name="agg", bufs=4,
                                             space="PSUM"))

        # ---- constants ----
        iota_i = cp.tile([P, P], mybir.dt.int32)
        nc.gpsimd.iota(iota_i[:], pattern=[[1, P]], base=0,
                       channel_multiplier=0)
        iota_b = cp.tile([P, P], BF16)
        nc.vector.tensor_copy(iota_b[:], iota_i[:])
        ident_b = cp.tile([P, P], BF16)
        make_identity(nc, ident_b[:])
        ones_row = cp.tile([1, P], BF16)
        nc.gpsimd.memset(ones_row[:], 1.0)
        zeros_t = cp.tile([P, d_in], F32)
        nc.gpsimd.memset(zeros_t[:], 0.0)

        w1_sb = cp.tile([d_in, d_h], BF16)
        nc.sync.dma_start(w1_sb[:], w1_d[:, :])
        wc_sb = cp.tile([d_h, d_o], BF16)
        nc.sync.dma_start(wc_sb[:], wc_d[:, :])
        b1r = cp.tile([1, d_h], BF16)
        nc.sync.dma_start(b1r[:], b1_d[:, :])
        bcr = cp.tile([1, d_o], BF16)
        nc.sync.dma_start(bcr[:], bc_d[:, :])
        d2_sb = cp.tile([P, T], F32)
        nc.sync.dma_start(d2_sb[:], d2_d[:, :])
        dl_sb = cp.tile([P, plan.NCH], BF16)
        nc.sync.dma_start(dl_sb[:], dl_d[:, :])
        w_sb = cp.tile([P, plan.NCH], BF16)
        nc.sync.dma_start(w_sb[:], w_d[:, :])
        idx_sb = cp.tile([P, plan.IDXCOLS], I16)
        nc.sync.dma_start(idx_sb[:], idx_d[:, :])

        z2s_sb = bigp.tile([P, T, d_o], F32)

        def oh_build(qq, k, L, dtype):
            """Weighted one-hots for chunks [k*cpc, k*cpc+L) of quarter
            qq: oh[p, j, d] = (d == dl[p, col]) * w[p, col]."""
            oh = ohp.tile([P, cpc, P], dtype, tag=f"oh{qq}")
            c0 = int(plan.qcol0[qq]) + k * cpc
            iota_bc = iota_b[:].unsqueeze(1).broadcast_to([P, L, P])
            dl_bc = dl_sb[:, c0:c0 + L].unsqueeze(2).broadcast_to([P, L, P])
            w_bc = w_sb[:, c0:c0 + L].unsqueeze(2).broadcast_to([P, L, P])
            nc.vector.tensor_tensor(oh[:, :L, :], iota_bc, dl_bc,
                                    OP.is_equal)
            nc.vector.tensor_tensor(oh[:, :L, :], oh[:, :L, :], w_bc,
                                    OP.mult)
            return oh

        def agg_pass(table, d_f, f32_src, on_tile):
            """Per dst tile t: psum[P, d_f] = sum_e w_e * table[src_e],
            then on_tile(t, psum_or_None)."""
            calls = {}

            def get_call(qq, k):
                if (qq, k) not in calls:
                    L = min(cpc, int(plan.NQC[qq]) - k * cpc)
                    i0 = (int(plan.qcol0[qq]) + k * cpc) * 8
                    q0 = qq * QR
                    q1 = min(n, q0 + QR)
                    if f32_src:
                        g = gp.tile([P, cpc, d_f], F32, tag=f"g{qq}")
                    else:
                        g = gp.tile([P, cpc, d_f], BF16, tag=f"g{qq}")
                    nc.gpsimd.dma_gather(
                        out_ap=g[:, :L, :],
                        in_ap=table[q0:q1, :],
                        idxs_ap=idx_sb[:, i0:i0 + L * 8],
                        num_idxs=L * P,
                        num_idxs_reg=L * P,
                        elem_size=d_f,
                        single_packet=False,
                    )
                    if f32_src:
                        gb = gbp.tile([P, cpc, d_f], BF16, tag=f"gb{qq}")
                        nc.scalar.activation(gb[:, :L, :], g[:, :L, :],
                                             AF.Copy)
                        g = gb
                    oh = oh_build(qq, k, L, BF16)
                    calls[(qq, k)] = (g, oh)
                return calls[(qq, k)]

            for t in range(T):
                ops = [(qq, j) for qq in range(NQ)
                       for j in range(int(plan.chunk0[qq, t]),
                                      int(plan.chunk0[qq, t])
                                      + int(plan.nch_qt[qq, t]))]
                if not ops:
                    on_tile(t, None)
                    continue
                psa = aggp.tile([P, d_f], F32, space="PSUM", tag="agg", bufs=3)
                for i, (qq, j) in enumerate(ops):
                    g, oh = get_call(qq, j // cpc)
                    col = j % cpc
                    nc.tensor.matmul(psa[:], lhsT=oh[:, col, :],
                                     rhs=g[:, col, :],
                                     start=(i == 0),
                                     stop=(i == len(ops) - 1))
                on_tile(t, psa)

        # ---- pass 1: agg_x[t] = sum w_e x[src]; project + self ----
        def tile1(t, psa):
            xo = xop.tile([P, d_in], BF16, tag="xo")
            nc.sync.dma_start(xo[:], xown_d[t * P:(t + 1) * P, :])
            tmp = tp.tile([P, d_in], BF16, tag="tmp")
            nc.vector.scalar_tensor_tensor(
                tmp[:], xo[:], d2_sb[:, t:t + 1],
                psa[:] if psa is not None else zeros_t[:],
                OP.mult, OP.add)
            pst = mmp.tile([P, P], BF16, space="PSUM", tag="mmb", bufs=2)
            nc.tensor.transpose(pst[:d_in, :], tmp[:], ident_b[:])
            aggT = tp.tile([d_in, P], BF16, tag="aggT")
            nc.scalar.activation(aggT[:], pst[:d_in, :], AF.Copy)
            psh = mmp.tile([P, P], F32, space="PSUM", tag="mm", bufs=3)
            nc.tensor.matmul(psh[:, :d_h], lhsT=ones_row[:], rhs=b1r[:],
                             start=True, stop=False)
            nc.tensor.matmul(psh[:, :d_h], lhsT=aggT[:], rhs=w1_sb[:],
                             start=False, stop=True)
            hr = tp.tile([P, d_h], BF16, tag="hr")
            nc.scalar.activation(hr[:], psh[:, :d_h], AF.Relu)
            psht = mmp.tile([P, P], BF16, space="PSUM", tag="mmb", bufs=2)
            nc.tensor.transpose(psht[:d_h, :], hr[:], ident_b[:])
            hT = tp.tile([d_h, P], BF16, tag="hT")
            nc.scalar.activation(hT[:], psht[:d_h, :], AF.Copy)
            psz = mmp.tile([P, P], F32, space="PSUM", tag="mm", bufs=3)
            nc.tensor.matmul(psz[:, :d_o], lhsT=hT[:], rhs=wc_sb[:],
                             start=True, stop=True)
            nc.vector.tensor_copy(z2s_sb[:, t, :], psz[:, :d_o])
            r0 = t * P
            r1 = min(S, r0 + P)
            if r1 > r0:
                nc.sync.dma_start(z2s_own[r0:r1, :], z2s_sb[:r1 - r0, t, :])

        if _stage in (1, 2):
            agg_pass(xtab_d, d_in, False, tile1)
        else:
            for t in range(T):
                tile1(t, None)

        if _nocoll:
            nc.sync.dma_start(z2s_full[0:S, :], z2s_own[:, :])
        else:
            nc.gpsimd.collective_compute(
                "AllGather", mybir.AluOpType.bypass, replica_groups=rg,
                ins=[z2s_own[:, :].opt()], outs=[z2s_full[:, :].opt()])

        # ---- pass 2: out[t] = bcat + sum w_e z2[src] + dinv2 z2own ----
        def tile2(t, psa):
            if psa is None:
                psa = aggp.tile([P, d_o], F32, space="PSUM", tag="agg", bufs=3)
                nc.tensor.matmul(psa[:], lhsT=ones_row[:],
                                 rhs=bcr[:], start=True, stop=True)
            o = tp.tile([P, d_o], F32, tag="o")
            nc.vector.scalar_tensor_tensor(
                o[:], z2s_sb[:, t, :], d2_sb[:, t:t + 1], psa[:],
                OP.mult, OP.add)
            nc.sync.dma_start(out_d[t * P:(t + 1) * P, :], o[:])

        if _stage in (2, 3):
            def agg2(table, d_f):
                calls = {}

                def get_call(qq, k):
                    if (qq, k) not in calls:
                        L = min(cpc, int(plan.NQC[qq]) - k * cpc)
                        i0 = (int(plan.qcol0[qq]) + k * cpc) * 8
                        q0 = qq * QR
                        q1 = min(n, q0 + QR)
                        g = gp.tile([P, cpc, d_f], F32, tag=f"g{qq}")
                        nc.gpsimd.dma_gather(
                            out_ap=g[:, :L, :],
                            in_ap=table[q0:q1, :],
                            idxs_ap=idx_sb[:, i0:i0 + L * 8],
                            num_idxs=L * P,
                            num_idxs_reg=L * P,
                            elem_size=d_f,
                            single_packet=False,
                        )
                        gb = gbp.tile([P, cpc, d_f], BF16, tag=f"gb{qq}")
                        nc.scalar.activation(gb[:, :L, :], g[:, :L, :],
                                             AF.Copy)
                        oh = oh_build(qq, k, L, BF16)
                        calls[(qq, k)] = (gb, oh)
                    return calls[(qq, k)]

                for t in range(T):
                    ops = [(qq, j) for qq in range(NQ)
                           for j in range(int(plan.chunk0[qq, t]),
                                          int(plan.chunk0[qq, t])
                                          + int(plan.nch_qt[qq, t]))]
                    psa = aggp.tile([P, d_f], F32, space="PSUM",
                                    tag="agg", bufs=3)
                    nc.tensor.matmul(psa[:], lhsT=ones_row[:],
                                     rhs=bcr[:], start=True,
                                     stop=not ops)
                    for i, (qq, j) in enumerate(ops):
                        g, oh = get_call(qq, j // cpc)
                        col = j % cpc
                        nc.tensor.matmul(psa[:], lhsT=oh[:, col, :],
                                         rhs=g[:, col, :],
                                         start=False,
                                         stop=(i == len(ops) - 1))
                    tile2(t, psa)

            agg2(z2s_full, d_o)
        else:
            for t in range(T):
                tile2(t, None)

    nc.compile()
    return nc


_CACHE = {}


def _get_program(n, e, d_in, d_h, d_o, n_cores, cpc, edge_key, src, dst):
    key = (n, e, d_in, d_h, d_o, n_cores, cpc, edge_key)
    if key not in _CACHE:
        plan = _Plan(n, n_cores, cpc, src, dst)
        nc = _build(plan, d_in, d_h, d_o)
        _CACHE[key] = (plan, nc)
    return _CACHE[key]


def kernel(x, edge_index, W1, b1, W_mu, b_mu, W_log, b_log,
           n_cores=8, cpc=16, _run_kwargs=None):
    from concourse.bass_utils import run_bass_kernel_spmd

    x = np.asarray(x, np.float32)
    edge_index = np.asarray(edge_index)
    W1 = np.asarray(W1, np.float32)
    Wcat = np.concatenate([np.asarray(W_mu, np.float32),
                           np.asarray(W_log, np.float32)], axis=1)
    bcat = np.concatenate([np.asarray(b_mu, np.float32),
                           np.asarray(b_log, np.float32)])
    b1 = np.asarray(b1, np.float32)
    n, d_in = x.shape
    d_h = W1.shape[1]
    d_o = Wcat.shape[1]
    lat = np.asarray(W_mu, np.float32).shape[1]
    src = edge_index[0].astype(np.int64)
    dst = edge_index[1].astype(np.int64)

    edge_key = hash((src.tobytes(), dst.tobytes()))
    plan, nc = _get_program(n, len(src), d_in, d_h, d_o, n_cores, cpc,
                            edge_key, src, dst)

    deg = np.bincount(dst, minlength=n)
    dinv = (1.0 / np.sqrt(1.0 + deg)).astype(np.float32)
    xtab = x.astype(BF)
    in_maps = []
    for c in range(n_cores):
        idx_u, dl, w, d2 = plan.core_arrays(c, dinv)
        xown = np.zeros((plan.SPAD, d_in), BF)
        xown[:plan.S] = xtab[c * plan.S:(c + 1) * plan.S]
        in_maps.append({
            "xtab": xtab, "xown": xown,
            "w1": W1.astype(BF), "wcat": Wcat.astype(BF),
            "b1": b1.astype(BF)[None, :], "bcat": bcat.astype(BF)[None, :],
            "dinv2": d2, "dl": dl, "w": w, "gidx": idx_u,
        })

    global _LAST_RESULT, _LAST_IN_MAPS
    _LAST_IN_MAPS = in_maps
    res = run_bass_kernel_spmd(nc, in_maps, core_ids=list(range(n_cores)),
                               **(_run_kwargs or {}))
    _LAST_RESULT = res
    out = np.concatenate(
        [res.results[c]["out2"][:plan.S] for c in range(n_cores)], axis=0)
    return (out[:, :lat].copy(), out[:, lat:].copy())


_LAST_RESULT = None
_LAST_IN_MAPS = None


# revision 10
# speedup vs baseline: 6.4114x; 3.5946x over previous
"""GCN VGAE encoder (3x GCNConv) on 8 Trainium2 NeuronCores.

Strategy: shard nodes across 8 cores, partition edges by destination
node (host-side 1D graph partitioning), replicate weights.

Math: A_hat = D^-1/2 (A+I) D^-1/2, dinv = 1/sqrt(deg+1). Aggregation
commutes with the linear projections, so layer 1 aggregates RAW x rows
(gathered bf16 from a replicated DRAM table -- no AllGather needed) and
projects afterwards. The per-edge norm w_e = dinv[src]*dinv[dst] is
folded into the one-hot scatter weights (host-computed), and the self
loop contributes dinv^2 * row analytically (host-precomputed scaled
transposed x). Layer-1 chunk matmuls run "transposed" (lhsT=messages,
rhs=one-hot) so the aggregate lands feature-major: the projections then
use stationary weights and fused bias+relu, with a single PE transpose
per tile to get z2 back to node-major for the gather table. Layers 2/3
share the adjacency, so W_mu|W_log fuse into one 64-wide pass over the
z2 table (f32 DRAM for the 256B gather element constraint, cast to bf16
on-chip for 4x PE throughput). One-hot matrices are built 16 chunks per
DVE instruction via stride-0 broadcast APs. Gather calls round-robin
over multiple SWDGE queues (a single queue drains at ~1 SDMA engine's
bandwidth).
"""

import numpy as np
import ml_dtypes

P = 128
BF = ml_dtypes.bfloat16


def _ceil_div(a, b):
    return -(-a // b)


class _Plan:
    """Host-side edge partitioning shared by all cores (SPMD => one
    common padded chunk structure = max over cores).

    Edges are grouped by (dst core, src quarter, dst tile); each (q,t)
    run is padded to a multiple of 128 so chunks never span tiles.
    Quarter q's edge stream is the concatenation of its runs over t.
    """

    def __init__(self, n, n_cores, cpc, src, dst, qr=25000):
        assert n % n_cores == 0
        self.n = n
        self.n_cores = n_cores
        self.cpc = cpc
        self.S = n // n_cores
        self.T = _ceil_div(self.S, P)
        self.SPAD = self.T * P
        self.NQ = _ceil_div(n, qr)
        self.QR = qr

        core = dst // self.S
        drel = dst - core * self.S
        tt = drel // P
        loc = (drel % P).astype(np.float32)
        q = src // qr
        qsrc = (src - q * qr).astype(np.int16)

        T, NQ = self.T, self.NQ
        counts = np.bincount((core * NQ + q) * T + tt,
                             minlength=n_cores * NQ * T).reshape(
            n_cores, NQ, T)
        rl = counts.max(axis=0)                       # [NQ, T]
        self.nch_qt = _ceil_div(rl, P)                # chunks per (q,t)
        PL = self.nch_qt * P                          # padded run length
        qt_start = np.zeros((NQ, T), np.int64)
        qt_start[:, 1:] = np.cumsum(PL, axis=1)[:, :-1]
        self.qt_start = qt_start
        self.NQC = PL.sum(axis=1) // P                # chunks per quarter
        self.NCH = int(self.NQC.sum())
        self.qcol0 = np.zeros(NQ + 1, np.int64)
        self.qcol0[1:] = np.cumsum(self.NQC)
        self.chunk0 = qt_start // P                   # [NQ, T]

        # order edges by (core, quarter, tile, src) -- src-sorted runs
        # give the gather DMA ascending addresses (DRAM locality).
        sidx = np.lexsort((src, tt, q, core))
        self.sc = core[sidx]
        self.sq = q[sidx]
        self.st = tt[sidx]
        self.sqsrc = qsrc[sidx]
        self.sloc = loc[sidx]
        self.ssrc = src[sidx]
        self.sdst = dst[sidx]
        gkey = (self.sc * NQ + self.sq) * T + self.st
        first = np.r_[True, gkey[1:] != gkey[:-1]]
        gstart = np.flatnonzero(first)
        glen = np.diff(np.r_[gstart, len(gkey)])
        self.rank = np.arange(len(gkey)) - np.repeat(gstart, glen)

        self.ncalls = [_ceil_div(int(c), cpc) for c in self.NQC]
        self.IDXCOLS = self.NCH * 8

    def core_arrays(self, c, dinv):
        """Per-core tensors: gather idx [P, IDXCOLS] i16, dl [P, NCH]
        bf16, w [P, NCH] bf16, dinv2 [SPAD] f32."""
        NQ, NCH = self.NQ, self.NCH
        m = self.sc == c
        mq = self.sq[m]
        mt = self.st[m]
        mrank = self.rank[m]
        msrc = self.sqsrc[m]
        mloc = self.sloc[m]
        mw = (dinv[self.ssrc[m]] * dinv[self.sdst[m]]).astype(np.float32)

        pos = self.qt_start[mq, mt] + mrank           # slot in q stream
        ccol = self.qcol0[mq] + pos // P              # global chunk col
        dl = np.full((NCH, P), 255.0, np.float32)
        w = np.zeros((NCH, P), np.float32)
        dl[ccol, pos % P] = mloc
        w[ccol, pos % P] = mw

        idx_out = np.zeros((P, self.IDXCOLS), np.int16)
        for qq in range(NQ):
            nqc = int(self.NQC[qq])
            if nqc == 0:
                continue
            arr = np.zeros(nqc * P, np.int16)
            mm = mq == qq
            arr[pos[mm]] = msrc[mm]
            c0 = int(self.qcol0[qq]) * 8
            wrapped = arr.reshape(nqc * 8, 16).T      # [16, nqc*8]
            idx_out[:, c0:c0 + nqc * 8] = np.tile(wrapped, (8, 1))

        d2 = np.zeros(self.SPAD, np.float32)
        d2[:self.S] = dinv[c * self.S:(c + 1) * self.S] ** 2
        return (idx_out, dl.T.astype(BF).copy(), w.T.astype(BF).copy(), d2)


def _build(plan, d_in, d_h, d_o):
    """Build the SPMD Bass program (same for every core)."""
    import concourse.mybir as mybir
    import concourse.tile as tile
    from concourse import bacc
    from concourse.masks import make_identity
    from contextlib import ExitStack
    import os

    F32 = mybir.dt.float32
    BF16 = mybir.dt.bfloat16
    I16 = mybir.dt.int16
    AF = mybir.ActivationFunctionType
    OP = mybir.AluOpType
    n, T, NQ, SPAD, S, QR = plan.n, plan.T, plan.NQ, plan.SPAD, plan.S, plan.QR
    cpc = plan.cpc
    n_cores = plan.n_cores
    _stage = int(os.environ.get("GCN_STAGE", "2"))
    _nocoll = bool(os.environ.get("GCN_NOCOLL"))
    _nq = int(os.environ.get("GCN_QUEUES", "4"))

    nc = bacc.Bacc("TRN2", target_bir_lowering=False,
                   debug=False, num_devices=n_cores, num_swdge_queues=_nq)

    xtab_d = nc.dram_tensor("xtab", [n, d_in], BF16, kind="ExternalInput")
    xst_d = nc.dram_tensor("xst", [P, T * P], BF16, kind="ExternalInput")
    w1_d = nc.dram_tensor("w1", [d_in, d_h], BF16, kind="ExternalInput")
    wc_d = nc.dram_tensor("wcat", [d_h, d_o], BF16, kind="ExternalInput")
    b1_d = nc.dram_tensor("b1", [d_h, 1], F32, kind="ExternalInput")
    bc_d = nc.dram_tensor("bcat", [1, d_o], BF16, kind="ExternalInput")
    d2_d = nc.dram_tensor("dinv2", [P, T], F32, kind="ExternalInput")
    dl_d = nc.dram_tensor("dl", [P, plan.NCH], BF16, kind="ExternalInput")
    w_d = nc.dram_tensor("w", [P, plan.NCH], BF16, kind="ExternalInput")
    idx_d = nc.dram_tensor("gidx", [P, plan.IDXCOLS], I16,
                           kind="ExternalInput")
    out_d = nc.dram_tensor("out2", [SPAD, d_o], F32, kind="ExternalOutput")

    z2s_own = nc.dram_tensor("z2s_own", [S, d_o], F32, kind="Internal")
    z2s_full = nc.dram_tensor("z2s_full", [n, d_o], F32, kind="Internal",
                              addr_space="Shared")
    rg = [list(range(n_cores))]

    qctr = [0]

    def next_queue():
        q = qctr[0] % _nq
        qctr[0] += 1
        return q

    with tile.TileContext(nc, num_cores=n_cores) as tc, ExitStack() as st:
        cp = st.enter_context(tc.tile_pool(name="consts", bufs=1))
        bigp = st.enter_context(tc.tile_pool(name="big", bufs=1))
        gp = st.enter_context(tc.tile_pool(name="gath", bufs=3))
        gbp = st.enter_context(tc.tile_pool(name="gbf", bufs=3))
        ohp = st.enter_context(tc.tile_pool(name="oh", bufs=3))
        tp = st.enter_context(tc.tile_pool(name="stage", bufs=4))
        mmp = st.enter_context(tc.tile_pool(name="mm", bufs=4, space="PSUM"))
        aggp = st.enter_context(tc.tile_pool(name="agg", bufs=4,
                                             space="PSUM"))

        # ---- constants ----
        iota_i = cp.tile([P, P], mybir.dt.int32)
        nc.gpsimd.iota(iota_i[:], pattern=[[1, P]], base=0,
                       channel_multiplier=0)
        iota_b = cp.tile([P, P], BF16)
        nc.vector.tensor_copy(iota_b[:], iota_i[:])
        ident_b = cp.tile([P, P], BF16)
        make_identity(nc, ident_b[:])
        ones_row = cp.tile([1, P], BF16)
        nc.gpsimd.memset(ones_row[:], 1.0)
        zeros_t = cp.tile([P, d_in], F32)
        nc.gpsimd.memset(zeros_t[:], 0.0)

        w1_sb = cp.tile([d_in, d_h], BF16)
        nc.sync.dma_start(w1_sb[:], w1_d[:, :])
        wc_sb = cp.tile([d_h, d_o], BF16)
        nc.sync.dma_start(wc_sb[:], wc_d[:, :])
        b1c = cp.tile([d_h, 1], F32)
        nc.sync.dma_start(b1c[:], b1_d[:, :])
        bcr = cp.tile([1, d_o], BF16)
        nc.sync.dma_start(bcr[:], bc_d[:, :])
        d2_sb = cp.tile([P, T], F32)
        nc.sync.dma_start(d2_sb[:], d2_d[:, :])
        dl_sb = cp.tile([P, plan.NCH], BF16)
        nc.sync.dma_start(dl_sb[:], dl_d[:, :])
        w_sb = cp.tile([P, plan.NCH], BF16)
        nc.sync.dma_start(w_sb[:], w_d[:, :])
        idx_sb = cp.tile([P, plan.IDXCOLS], I16)
        nc.sync.dma_start(idx_sb[:], idx_d[:, :])

        # bcat broadcast to all partitions via ones-matmul
        psb = mmp.tile([P, P], F32, space="PSUM", tag="mm", bufs=3)
        nc.tensor.matmul(psb[:, :d_o], lhsT=ones_row[:], rhs=bcr[:],
                         start=True, stop=True)
        bcbc = cp.tile([P, d_o], F32)
        nc.vector.tensor_copy(bcbc[:], psb[:, :d_o])

        xst_sb = bigp.tile([P, T, P], BF16)
        nc.sync.dma_start(xst_sb[:, :, :], xst_d[:, :])
        z2s_sb = bigp.tile([P, T, d_o], F32)

        def oh_build(qq, k, L):
            """Weighted one-hots for chunks [k*cpc, k*cpc+L) of quarter
            qq: oh[p, j, d] = (d == dl[p, col]) * w[p, col]."""
            oh = ohp.tile([P, cpc, P], BF16, tag=f"oh{qq}")
            c0 = int(plan.qcol0[qq]) + k * cpc
            iota_bc = iota_b[:].unsqueeze(1).broadcast_to([P, L, P])
            dl_bc = dl_sb[:, c0:c0 + L].unsqueeze(2).broadcast_to([P, L, P])
            w_bc = w_sb[:, c0:c0 + L].unsqueeze(2).broadcast_to([P, L, P])
            nc.vector.tensor_tensor(oh[:, :L, :], iota_bc, dl_bc,
                                    OP.is_equal)
            nc.vector.tensor_tensor(oh[:, :L, :], oh[:, :L, :], w_bc,
                                    OP.mult)
            return oh

        def gather_call(table, qq, k, d_f, dtype):
            L = min(cpc, int(plan.NQC[qq]) - k * cpc)
            i0 = (int(plan.qcol0[qq]) + k * cpc) * 8
            q0 = qq * QR
            q1 = min(n, q0 + QR)
            g = gp.tile([P, cpc, d_f], dtype, tag=f"g{qq}")
            nc.gpsimd.dma_gather(
                out_ap=g[:, :L, :],
                in_ap=table[q0:q1, :],
                idxs_ap=idx_sb[:, i0:i0 + L * 8],
                num_idxs=L * P,
                num_idxs_reg=L * P,
                elem_size=d_f,
                single_packet=False,
                queue_num=next_queue(),
            )
            return g, L

        def tile_ops(t):
            return [(qq, j) for qq in range(NQ)
                    for j in range(int(plan.chunk0[qq, t]),
                                   int(plan.chunk0[qq, t])
                                   + int(plan.nch_qt[qq, t]))]

        # ---- pass 1 (transposed): psum1T[f, d] = sum_e x[src_e]w_e ----
        def tile1(t, psa):
            agg1T = tp.tile([P, P], BF16, tag="aggT")
            if psa is not None:
                nc.vector.tensor_tensor(agg1T[:], psa[:],
                                        xst_sb[:, t, :], OP.add)
            else:
                nc.vector.tensor_copy(agg1T[:], xst_sb[:, t, :])
            psh = mmp.tile([P, P], F32, space="PSUM", tag="mm", bufs=3)
            nc.tensor.matmul(psh[:d_h, :], lhsT=w1_sb[:], rhs=agg1T[:],
                             start=True, stop=True)
            hT = tp.tile([d_h, P], BF16, tag="hT")
            nc.scalar.activation(hT[:], psh[:d_h, :], AF.Relu, bias=b1c[:])
            psz = mmp.tile([P, P], F32, space="PSUM", tag="mm", bufs=3)
            nc.tensor.matmul(psz[:d_o, :], lhsT=wc_sb[:], rhs=hT[:],
                             start=True, stop=True)
            z2T = tp.tile([d_o, P], BF16, tag="z2T")
            nc.vector.tensor_copy(z2T[:], psz[:d_o, :])
            psn = mmp.tile([P, P], BF16, space="PSUM", tag="mmb", bufs=2)
            nc.tensor.transpose(psn[:, :d_o], z2T[:], ident_b[:d_o, :d_o])
            nc.vector.tensor_copy(z2s_sb[:, t, :], psn[:, :d_o])
            r0 = t * P
            r1 = min(S, r0 + P)
            if r1 > r0:
                nc.sync.dma_start(z2s_own[r0:r1, :], z2s_sb[:r1 - r0, t, :])

        if _stage in (1, 2):
            calls = {}
            for t in range(T):
                ops = tile_ops(t)
                if not ops:
                    tile1(t, None)
                    continue
                psa = aggp.tile([P, P], F32, space="PSUM", tag="agg",
                                bufs=3)
                for i, (qq, j) in enumerate(ops):
                    k = j // cpc
                    if (qq, k) not in calls:
                        g, L = gather_call(xtab_d, qq, k, d_in, BF16)
                        calls[(qq, k)] = (g, oh_build(qq, k, L))
                    g, oh = calls[(qq, k)]
                    col = j % cpc
                    nc.tensor.matmul(psa[:], lhsT=g[:, col, :],
                                     rhs=oh[:, col, :],
                                     start=(i == 0),
                                     stop=(i == len(ops) - 1))
                tile1(t, psa)
        else:
            for t in range(T):
                tile1(t, None)

        if _nocoll:
            nc.sync.dma_start(z2s_full[0:S, :], z2s_own[:, :])
        else:
            nc.gpsimd.collective_compute(
                "AllGather", mybir.AluOpType.bypass, replica_groups=rg,
                ins=[z2s_own[:, :].opt()], outs=[z2s_full[:, :].opt()])

        # ---- pass 2: out[t] = bcat + sum w_e z2[src] + dinv2 z2own ----
        def tile2(t, psa):
            o = tp.tile([P, d_o], F32, tag="o")
            nc.vector.scalar_tensor_tensor(
                o[:], z2s_sb[:, t, :], d2_sb[:, t:t + 1],
                psa[:] if psa is not None else zeros_t[:, :d_o],
                OP.mult, OP.add)
            nc.vector.tensor_tensor(o[:], o[:], bcbc[:], OP.add)
            nc.sync.dma_start(out_d[t * P:(t + 1) * P, :], o[:])

        if _stage in (2, 3):
            calls = {}
            for t in range(T):
                ops = tile_ops(t)
                if not ops:
                    tile2(t, None)
                    continue
                psa = aggp.tile([P, d_o], F32, space="PSUM", tag="agg",
                                bufs=3)
                for i, (qq, j) in enumerate(ops):
                    k = j // cpc
                    if (qq, k) not in calls:
                        g, L = gather_call(z2s_full, qq, k, d_o, F32)
                        gb = gbp.tile([P, cpc, d_o], BF16, tag=f"gb{qq}")
                        nc.scalar.activation(gb[:, :L, :], g[:, :L, :],
                                             AF.Copy)
                        calls[(qq, k)] = (gb, oh_build(qq, k, L))
                    gb, oh = calls[(qq, k)]
                    col = j % cpc
                    nc.tensor.matmul(psa[:], lhsT=oh[:, col, :],
                                     rhs=gb[:, col, :],
                                     start=(i == 0),
                                     stop=(i == len(ops) - 1))
                tile2(t, psa)
        else:
            for t in range(T):
                tile2(t, None)

    nc.compile()
    return nc


_CACHE = {}


def _get_program(n, e, d_in, d_h, d_o, n_cores, cpc, edge_key, src, dst):
    key = (n, e, d_in, d_h, d_o, n_cores, cpc, edge_key)
    if key not in _CACHE:
        plan = _Plan(n, n_cores, cpc, src, dst)
        nc = _build(plan, d_in, d_h, d_o)
        _CACHE[key] = (plan, nc)
    return _CACHE[key]


def kernel(x, edge_index, W1, b1, W_mu, b_mu, W_log, b_log,
           n_cores=8, cpc=16, _run_kwargs=None):
    from concourse.bass_utils import run_bass_kernel_spmd

    x = np.asarray(x, np.float32)
    edge_index = np.asarray(edge_index)
    W1 = np.asarray(W1, np.float32)
    Wcat = np.concatenate([np.asarray(W_mu, np.float32),
                           np.asarray(W_log, np.float32)], axis=1)
    bcat = np.concatenate([np.asarray(b_mu, np.float32),
                           np.asarray(b_log, np.float32)])
    b1 = np.asarray(b1, np.float32)
    n, d_in = x.shape
    d_h = W1.shape[1]
    d_o = Wcat.shape[1]
    lat = np.asarray(W_mu, np.float32).shape[1]
    src = edge_index[0].astype(np.int64)
    dst = edge_index[1].astype(np.int64)

    edge_key = hash((src.tobytes(), dst.tobytes()))
    plan, nc = _get_program(n, len(src), d_in, d_h, d_o, n_cores, cpc,
                            edge_key, src, dst)

    deg = np.bincount(dst, minlength=n)
    dinv = (1.0 / np.sqrt(1.0 + deg)).astype(np.float32)
    xtab = x.astype(BF)
    in_maps = []
    for c in range(n_cores):
        idx_u, dl, w, d2 = plan.core_arrays(c, dinv)
        # xst[f, t*P + p] = dinv2[t*P+p] * x[c*S + t*P + p, f]
        xpad = np.zeros((plan.SPAD, d_in), np.float32)
        xpad[:plan.S] = x[c * plan.S:(c + 1) * plan.S]
        xst = (xpad * d2[:, None]).T.astype(BF).copy()
        in_maps.append({
            "xtab": xtab, "xst": xst,
            "w1": W1.astype(BF), "wcat": Wcat.astype(BF),
            "b1": b1.astype(np.float32)[:, None],
            "bcat": bcat.astype(BF)[None, :],
            "dinv2": d2.reshape(plan.T, P).T.copy(),
            "dl": dl, "w": w, "gidx": idx_u,
        })

    global _LAST_RESULT, _LAST_IN_MAPS
    _LAST_IN_MAPS = in_maps
    res = run_bass_kernel_spmd(nc, in_maps, core_ids=list(range(n_cores)),
                               **(_run_kwargs or {}))
    _LAST_RESULT = res
    out = np.concatenate(
        [res.results[c]["out2"][:plan.S] for c in range(n_cores)], axis=0)
    return (out[:, :lat].copy(), out[:, lat:].copy())


_LAST_RESULT = None
_LAST_IN_MAPS = None
